# revision 1
# baseline (speedup 1.0000x reference)
"""Causal self-attention Trainium2 Bass kernel (8 NeuronCores).

Problem: B=2, T=4096, C=512, H=8 heads, D=64 head dim.
  qkv = x @ w_attn.T + b_attn ; causal softmax attention ; y @ w_proj.T + b_proj

Sharding: 16 (batch, head) units over 8 cores -> each core handles one batch
and two adjacent heads (core = b*4 + hp, heads 2hp and 2hp+1). Weights are
sliced per core on the host; each core computes a [C, T] partial of the
output projection for its batch (heads contribution); host sums the 4
partials per batch and transposes back.

On-device layout is fully transposed (dims x T) to avoid all transposes
except a cheap one for V:
  qT/kT [128, T] (2 heads x 64 dims stacked), scores S.T [j, i] per
  (128-key-block x 512-query-tile), softmax denominator via an appended
  ones-column on the V operand of the P@V matmul, unsafe softmax (no max
  subtraction; scores are O(N(0,1)) so exp never overflows), division by the
  denominator deferred until after the attention loop (reciprocal +
  GPSIMD partition-broadcast), projection emitted as out.T [C, T].
"""

import os
import numpy as np

import concourse.bacc as bacc
import concourse.tile as tile
import concourse.mybir as mybir
from concourse import bass_utils
from concourse.bass import AP

F32 = mybir.dt.float32
F32R = mybir.dt.float32r
AF = mybir.ActivationFunctionType

B, T, C = 2, 4096, 512
H, D = 8, 64
N_CORES = 8
TQ = 512          # query tile (i-tile)
TJ = 128          # key block (j-block)
NI = T // TQ      # 8 i-tiles
NJ = T // TJ      # 32 j-blocks

BF16 = mybir.dt.bfloat16
# matmul input dtype: float32r (fp32 storage, reduced mantissa in the PE,
# full rate). Measured same speed as bf16 on this problem but 13x more
# accurate. F32 = exact, 4x slower.
MM_DT = F32R


def _mm(ap):
    return ap


def _emit(nc, tc, ctx):
    xT = nc.dram_tensor("xT", [C, T], MM_DT, kind="ExternalInput").ap()
    wqkvT = nc.dram_tensor("wqkvT", [C, 384], MM_DT, kind="ExternalInput").ap()
    bqkv = nc.dram_tensor("bqkv", [128, 3], F32, kind="ExternalInput").ap()
    wpT = nc.dram_tensor("wpT", [128, C], MM_DT, kind="ExternalInput").ap()
    bp = nc.dram_tensor("bp", [128, 4], F32, kind="ExternalInput").ap()
    mask01 = nc.dram_tensor("mask01", [128, 128], F32, kind="ExternalInput").ap()
    ident = nc.dram_tensor("ident", [128, 128], F32, kind="ExternalInput").ap()
    ones = nc.dram_tensor("ones", [128, 1], MM_DT, kind="ExternalInput").ap()
    outT = nc.dram_tensor("outT", [C, T], F32, kind="ExternalOutput").ap()

    consts = ctx.enter_context(tc.tile_pool(name="consts", bufs=1))
    big = ctx.enter_context(tc.tile_pool(name="big", bufs=1))
    xt_pool = ctx.enter_context(tc.tile_pool(name="xt", bufs=12))
    vt_pool = ctx.enter_context(tc.tile_pool(name="vt", bufs=2))
    p_pool = ctx.enter_context(tc.tile_pool(name="pp", bufs=6))
    yn_pool = ctx.enter_context(tc.tile_pool(name="yn", bufs=6))
    dt_pool = ctx.enter_context(tc.tile_pool(name="dtp", bufs=2))
    dscr_pool = ctx.enter_context(tc.tile_pool(name="dscr", bufs=2, space="DRAM"))
    os_pool = ctx.enter_context(tc.tile_pool(name="osp", bufs=4))
    # PSUM: 8 banks total. ps_a (4 slots x 1 bank): S.T tiles, QKV chunks,
    # V transposes, out-proj. ps_o (4 slots): the four O' accumulators of the
    # current (half, head).
    ps_a = ctx.enter_context(tc.tile_pool(name="ps_a", bufs=4, space="PSUM"))
    ps_o = ctx.enter_context(tc.tile_pool(name="ps_o", bufs=4, space="PSUM"))

    # --- constants ---
    w_sb = consts.tile([128, 4, 384], MM_DT, name="w_sb")
    nc.sync.dma_start(out=w_sb, in_=wqkvT.rearrange("(c p) m -> p c m", p=128))
    # split per head so each lhsT sits at base partition 0 (matmul requires
    # lhsT and rhs to share the base partition; the rhs y tiles are at 0)
    wp0_sb = consts.tile([64, C], MM_DT, name="wp0_sb")
    nc.sync.dma_start(out=wp0_sb, in_=wpT[0:64, :])
    wp1_sb = consts.tile([64, C], MM_DT, name="wp1_sb")
    nc.sync.dma_start(out=wp1_sb, in_=wpT[64:128, :])
    bqkv_sb = consts.tile([128, 3], F32, name="bqkv_sb")
    nc.sync.dma_start(out=bqkv_sb, in_=bqkv)
    bp_sb = consts.tile([128, 4], F32, name="bp_sb")
    nc.sync.dma_start(out=bp_sb, in_=bp)
    mask_sb = consts.tile([128, 128], F32, name="mask_sb")
    nc.sync.dma_start(out=mask_sb, in_=mask01)
    id_sb = consts.tile([128, 128], F32, name="id_sb")
    nc.sync.dma_start(out=id_sb, in_=ident)

    qT_sb = big.tile([128, T], MM_DT, name="qT_sb")
    kT_sb = big.tile([128, T], MM_DT, name="kT_sb")
    # V in natural layout per 128-key block, with a ones column appended per
    # head: [j, 0:64]=v_h0, 64=ones, [65:129]=v_h1, 129=ones.
    v_all = big.tile([128, NJ, 130], MM_DT, name="v_all")
    ones_bcast = AP(tensor=ones.tensor, offset=0, ap=[[1, 128], [0, NJ], [0, 1]])
    nc.gpsimd.dma_start(out=v_all[:, :, 64:65], in_=ones_bcast)
    nc.gpsimd.dma_start(out=v_all[:, :, 129:130], in_=ones_bcast)

    # --- QKV projection (transposed layout) ---
    for t in range(NI):
        t0 = t * TQ
        xcs = []
        for c in range(4):
            xc = xt_pool.tile([128, TQ], MM_DT, name="xc", tag="xc")
            deng = nc.sync if c % 2 == 0 else nc.gpsimd
            deng.dma_start(out=xc, in_=xT[c * 128:(c + 1) * 128, t0:t0 + TQ])
            xcs.append(xc)
        for m in range(3):  # q, k, v rows of the sliced w_attn
            ps = ps_a.tile([128, TQ], F32, name="qkv_ps", tag="a")
            for c in range(4):
                nc.tensor.matmul(
                    ps,
                    lhsT=_mm(w_sb[:, c, m * 128:(m + 1) * 128]),
                    rhs=_mm(xcs[c]),
                    start=(c == 0),
                    stop=(c == 3),
                )
            if m == 0:
                # q scale (1/sqrt(D)) is folded into wqkvT/bqkv on the host
                nc.vector.tensor_scalar_add(qT_sb[:, t0:t0 + TQ], ps, bqkv_sb[:, 0:1])
            elif m == 1:
                nc.vector.tensor_scalar_add(kT_sb[:, t0:t0 + TQ], ps, bqkv_sb[:, 1:2])
            else:
                vt = vt_pool.tile([128, TQ], F32, name="vt", tag="vt")
                nc.vector.tensor_scalar_add(vt, ps, bqkv_sb[:, 2:3])
                for s in range(4):
                    n = t * 4 + s
                    tp = ps_a.tile([128, 128], F32, name="tp", tag="a")
                    # fp32 PE transpose (exact): [vdim, t]^T -> [t, vdim]
                    nc.tensor.transpose(tp, vt[:, s * 128:(s + 1) * 128], id_sb)
                    dst = v_all[:, n, :].rearrange("p (g e) -> p g e", g=2, e=65)[:, :, 0:64]
                    src = tp.rearrange("p (g e) -> p g e", g=2, e=64)
                    nc.vector.tensor_copy(dst, src)

    # --- attention (j-outer within each half of the query range) ---
    yns = {}
    for half in range(2):
        i_ts = [half * 4 + k for k in range(4)]
        for h in range(2):
            hr = slice(h * 64, (h + 1) * 64)
            o_ts = {i_t: ps_o.tile([128, TQ], F32, name="o_ps", tag="o")
                    for i_t in i_ts}
            for J in range((half + 1) * 16):
                j0 = J * TJ
                Id = j0 // TQ
                k_lhs = kT_sb[hr, j0:j0 + TJ]
                v_lhs = v_all[:, J, 0:65] if h == 0 else v_all[:, J, 65:130]
                for i_t in i_ts:
                    if i_t < Id:
                        continue
                    i0 = i_t * TQ
                    diag = (Id == i_t)
                    r = j0 - i0 if diag else 0
                    st = ps_a.tile([128, TQ], F32, name="st", tag="a")
                    nc.tensor.matmul(
                        st[:, r:TQ],
                        lhsT=_mm(k_lhs),
                        rhs=_mm(qT_sb[hr, i0 + r:i0 + TQ]),
                        start=True, stop=True,
                    )
                    pt = p_pool.tile([128, TQ], MM_DT, name="pt", tag="p")
                    nc.scalar.activation(pt[:, r:TQ], st[:, r:TQ], AF.Exp)
                    if diag:
                        nc.vector.tensor_mul(pt[:, r:r + 128], pt[:, r:r + 128], mask_sb)
                    nc.tensor.matmul(
                        o_ts[i_t][0:65, r:TQ],
                        lhsT=_mm(v_lhs),
                        rhs=_mm(pt[:, r:TQ]),
                        start=(J == 0),
                        stop=(J == i_t * 4 + 3),
                    )
            # softmax division: denominator (ones-column row, partition 64)
            # -> SBUF -> DRAM-bounce broadcast down to partitions 0:64 ->
            # reciprocal at base partition 0 (recip_approx_fast and
            # partition_broadcast are HW-broken at base partition != 0).
            for i_t in i_ts:
                o = o_ts[i_t]
                dt_t = dt_pool.tile([128, TQ], F32, name="dt_t", tag="dt")
                nc.vector.tensor_copy(dt_t[64:65, :], o[64:65, :])
                scr = dscr_pool.tile([1, TQ], F32, name="scr", tag="scr")
                nc.sync.dma_start(out=scr, in_=dt_t[64:65, :])
                bc = AP(tensor=scr.tensor, offset=scr.offset, ap=[[0, 64], [1, TQ]])
                nc.sync.dma_start(out=dt_t[0:64, :], in_=bc)
                rc = dt_pool.tile([64, TQ], F32, name="rc", tag="rc")
                nc.vector.reciprocal_approx_fast(out=rc, in_=dt_t[0:64, :])
                yn = yn_pool.tile([64, TQ], MM_DT, name=f"yn{h}", tag=f"yn{h}")
                nc.vector.tensor_mul(yn, o[0:64, :], rc)
                yns[(h, i_t)] = yn
        # --- output projection for this half (partial out.T, 2 heads) ---
        for i_t in i_ts:
            i0 = i_t * TQ
            for mc in range(4):
                po = ps_a.tile([128, TQ], F32, name="po", tag="a")
                nc.tensor.matmul(po, lhsT=_mm(wp0_sb[:, mc * 128:(mc + 1) * 128]),
                                 rhs=_mm(yns[(0, i_t)]), start=True, stop=False)
                nc.tensor.matmul(po, lhsT=_mm(wp1_sb[:, mc * 128:(mc + 1) * 128]),
                                 rhs=_mm(yns[(1, i_t)]), start=False, stop=True)
                ob = os_pool.tile([128, TQ], F32, name="ob", tag="os")
                nc.vector.tensor_scalar_add(ob, po, bp_sb[:, mc:mc + 1])
                nc.sync.dma_start(out=outT[mc * 128:(mc + 1) * 128, i0:i0 + TQ], in_=ob)


_CACHED_NC = None


def _build_program():
    global _CACHED_NC
    if _CACHED_NC is not None:
        return _CACHED_NC
    from contextlib import ExitStack
    nc = bacc.Bacc("TRN2", target_bir_lowering=False, debug=False,
                   num_devices=N_CORES)
    with tile.TileContext(nc) as tc:
        with ExitStack() as ctx:
            _emit(nc, tc, ctx)
    nc.compile()
    _CACHED_NC = nc
    return nc


def _host_inputs(x, w_attn, b_attn, w_proj, b_proj):
    """Build the 8 per-core input maps."""
    import ml_dtypes
    mmnp = np.float32 if MM_DT in (F32R, F32) else ml_dtypes.bfloat16
    x = np.asarray(x, dtype=np.float32)
    w_attn = np.asarray(w_attn, dtype=np.float32)
    b_attn = np.asarray(b_attn, dtype=np.float32)
    w_proj = np.asarray(w_proj, dtype=np.float32)
    b_proj = np.asarray(b_proj, dtype=np.float32)

    scale = np.float32(1.0 / np.sqrt(D))
    mask = np.triu(np.ones((128, 128), dtype=np.float32))  # keep jj <= ii
    ident = np.eye(128, dtype=np.float32)

    xT_b = [np.ascontiguousarray(x[b].T) for b in range(B)]

    in_maps = []
    for core in range(N_CORES):
        b, hp = divmod(core, 4)
        r0 = 2 * hp * 64  # first row of this core's head-pair slice
        qr = w_attn[r0:r0 + 128] * scale
        kr = w_attn[C + r0:C + r0 + 128]
        vr = w_attn[2 * C + r0:2 * C + r0 + 128]
        wqkvT = np.ascontiguousarray(np.concatenate([qr, kr, vr], axis=0).T)
        bq = b_attn[r0:r0 + 128] * scale
        bk = b_attn[C + r0:C + r0 + 128]
        bv = b_attn[2 * C + r0:2 * C + r0 + 128]
        bqkv = np.ascontiguousarray(np.stack([bq, bk, bv], axis=1))
        wpT = np.ascontiguousarray(w_proj[:, r0:r0 + 128].T)
        if hp == 0:
            bp = np.ascontiguousarray(b_proj.reshape(4, 128).T)
        else:
            bp = np.zeros((128, 4), dtype=np.float32)
        in_maps.append({
            "xT": xT_b[b].astype(mmnp),
            "wqkvT": wqkvT.astype(mmnp),
            "bqkv": bqkv,
            "wpT": wpT.astype(mmnp),
            "bp": bp,
            "mask01": mask,
            "ident": ident,
            "ones": np.ones((128, 1), dtype=mmnp),
        })
    return in_maps


def _gather(results):
    out = np.empty((B, T, C), dtype=np.float32)
    for b in range(B):
        acc = results[b * 4]["outT"].astype(np.float32).copy()
        for hp in range(1, 4):
            acc += results[b * 4 + hp]["outT"]
        out[b] = acc.T
    return out


def kernel(x, w_attn, b_attn, w_proj, b_proj, _run_kwargs=None):
    nc = _build_program()
    in_maps = _host_inputs(x, w_attn, b_attn, w_proj, b_proj)
    kw = dict(_run_kwargs or {})
    res = bass_utils.run_bass_kernel_spmd(nc, in_maps,
                                          core_ids=list(range(N_CORES)), **kw)
    out = _gather(res.results)
    if _run_kwargs is not None:
        kernel.last_result = res
    return out



# revision 4
# speedup vs baseline: 1.4574x; 1.4574x over previous
"""Causal self-attention Trainium2 Bass kernel (8 NeuronCores).

Problem: B=2, T=4096, C=512, H=8 heads, D=64 head dim.
  qkv = x @ w_attn.T + b_attn ; causal softmax attention ; y @ w_proj.T + b_proj

Sharding: 16 (batch, head) units over 8 cores -> core = b*4 + hp handles batch b
and heads 2hp, 2hp+1. Weights sliced per core on the host; each core emits a
[C, T] bf16 partial of the projected output for its head pair; the host sums
the 4 partials per batch (f32) and transposes back.

Design notes (v2):
- ACT (scalar engine) exp throughput is the hard floor (~1 elem/cycle/lane);
  the loop is arranged so ACT streams continuously: scores for two key-blocks
  are staged into one 2-bank PSUM tile and consumed by a single wide ACTIVATE
  (diag sub-blocks are packed contiguously so no garbage columns are read).
- All matmul lhsT operands span the full 128 partitions (per-head q is
  zero-padded into qT0/qT1) so LDWEIGHTS pipelines into the background weight
  buffer; partial-partition lhsT (row_grp) was measured to serialize
  LDWEIGHTS with the matmul stream and keep the PE HAM-throttled at 1.2 GHz.
- QKV projection is emitted just-in-time, interleaved between attention pairs
  (generator-driven) so there is no serial startup phase; x streams in as
  bf16 (halves HBM traffic).
- Softmax denominator rides as a ones-column in the P@V lhsT (M=65); the
  division avoids the DRAM bounce: evict O' to SBUF, matmul against a one-hot
  lhsT (e64) to broadcast the denominator row across 64 partitions,
  reciprocal, multiply.
- Both heads' normalized outputs are stacked into one [128, TQ] tile so the
  output projection runs with K=128 (half the matmuls); the result is stored
  as bf16 (halves the output DMA).
"""

import numpy as np

import concourse.bacc as bacc
import concourse.tile as tile
import concourse.mybir as mybir
from concourse import bass_utils
from concourse.bass import AP

F32 = mybir.dt.float32
F32R = mybir.dt.float32r
BF16 = mybir.dt.bfloat16
AF = mybir.ActivationFunctionType

B, T, C = 2, 4096, 512
H, D = 8, 64
N_CORES = 8
TQ = 512          # query tile
TJ = 128          # key block
NI = T // TQ      # 8 i-tiles
NJ = T // TJ      # 32 j-blocks

MM_DT = F32R      # attention-internal matmul dtype
IN_DT = BF16      # x / w_attn dtype (DMA-bound input path)


def _emit(nc, tc, ctx):
    xT = nc.dram_tensor("xT", [C, T], IN_DT, kind="ExternalInput").ap()
    wqkvT = nc.dram_tensor("wqkvT", [C, 384], IN_DT, kind="ExternalInput").ap()
    bqkv = nc.dram_tensor("bqkv", [128, 3], F32, kind="ExternalInput").ap()
    wpT = nc.dram_tensor("wpT", [128, C], MM_DT, kind="ExternalInput").ap()
    bp = nc.dram_tensor("bp", [128, 4], F32, kind="ExternalInput").ap()
    mask01 = nc.dram_tensor("mask01", [128, 128], F32, kind="ExternalInput").ap()
    ident = nc.dram_tensor("ident", [128, 128], F32, kind="ExternalInput").ap()
    ones = nc.dram_tensor("ones", [128, 1], MM_DT, kind="ExternalInput").ap()
    e64 = nc.dram_tensor("e64", [128, 64], MM_DT, kind="ExternalInput").ap()
    outT = nc.dram_tensor("outT", [C, T], BF16, kind="ExternalOutput").ap()

    consts = ctx.enter_context(tc.tile_pool(name="consts", bufs=1))
    big = ctx.enter_context(tc.tile_pool(name="big", bufs=1))
    xt_pool = ctx.enter_context(tc.tile_pool(name="xt", bufs=8))
    vt_pool = ctx.enter_context(tc.tile_pool(name="vt", bufs=2))
    pt_pool = ctx.enter_context(tc.tile_pool(name="pt", bufs=3))
    osb_pool = ctx.enter_context(tc.tile_pool(name="osb", bufs=3))
    rc_pool = ctx.enter_context(tc.tile_pool(name="rc", bufs=3))
    yn_pool = ctx.enter_context(tc.tile_pool(name="yn", bufs=6))
    ob_pool = ctx.enter_context(tc.tile_pool(name="ob", bufs=4))
    # PSUM: 8 banks = stage 2x[128,1024] (4) + o 2x[128,512] (2) + sm 2x (2)
    ps_stage = ctx.enter_context(tc.tile_pool(name="ps_st", bufs=2, space="PSUM"))
    ps_o = ctx.enter_context(tc.tile_pool(name="ps_o", bufs=2, space="PSUM"))
    ps_sm = ctx.enter_context(tc.tile_pool(name="ps_sm", bufs=2, space="PSUM"))

    # --- constants ---
    w_sb = consts.tile([128, 4, 384], IN_DT, name="w_sb")
    nc.sync.dma_start(out=w_sb, in_=wqkvT.rearrange("(c p) m -> p c m", p=128))
    wp_sb = consts.tile([128, C], MM_DT, name="wp_sb")
    nc.sync.dma_start(out=wp_sb, in_=wpT)
    bqkv_sb = consts.tile([128, 3], F32, name="bqkv_sb")
    nc.sync.dma_start(out=bqkv_sb, in_=bqkv)
    bp_sb = consts.tile([128, 4], F32, name="bp_sb")
    nc.sync.dma_start(out=bp_sb, in_=bp)
    mask_sb = consts.tile([128, 128], F32, name="mask_sb")
    nc.sync.dma_start(out=mask_sb, in_=mask01)
    id_sb = consts.tile([128, 128], F32, name="id_sb")
    nc.sync.dma_start(out=id_sb, in_=ident)
    e64_sb = consts.tile([128, 64], MM_DT, name="e64_sb")
    nc.sync.dma_start(out=e64_sb, in_=e64)

    # per-head q (zero-padded to 128 partitions), shared k, natural-layout v
    qT0 = big.tile([128, T], MM_DT, name="qT0")
    qT1 = big.tile([128, T], MM_DT, name="qT1")
    kT_sb = big.tile([128, T], MM_DT, name="kT_sb")
    nc.vector.memset(qT0[64:128, :].bitcast(mybir.dt.uint32), 0)
    nc.gpsimd.memset(qT1[0:64, :].bitcast(mybir.dt.uint32), 0)
    # V per 128-key block with ones columns: [j, 0:64]=v_h0, 64=ones,
    # [65:129]=v_h1, 129=ones
    v_all = big.tile([128, NJ, 130], MM_DT, name="v_all")
    ones_bcast = AP(tensor=ones.tensor, offset=0, ap=[[1, 128], [0, NJ], [0, 1]])
    nc.gpsimd.dma_start(out=v_all[:, :, 64:65], in_=ones_bcast)
    nc.gpsimd.dma_start(out=v_all[:, :, 129:130], in_=ones_bcast)

    # --- QKV projection for one i-tile (generator: yields between chunks) ---
    def emit_qkv(t):
        t0 = t * TQ
        xcs = []
        for c in range(4):
            xc = xt_pool.tile([128, TQ], IN_DT, name="xc", tag="xc")
            deng = nc.sync if c % 2 == 0 else nc.gpsimd
            deng.dma_start(out=xc, in_=xT[c * 128:(c + 1) * 128, t0:t0 + TQ])
            xcs.append(xc)
        yield
        for m in range(3):  # q, k, v
            ps = ps_sm.tile([128, TQ], F32, name="qkv_ps", tag="sm")
            for c in range(4):
                nc.tensor.matmul(
                    ps,
                    lhsT=w_sb[:, c, m * 128:(m + 1) * 128],
                    rhs=xcs[c],
                    start=(c == 0),
                    stop=(c == 3),
                )
            if m == 0:
                # q scale (1/sqrt(D)) folded into wqkvT/bqkv on the host
                nc.vector.tensor_scalar_add(
                    qT0[0:64, t0:t0 + TQ], ps[0:64, :], bqkv_sb[0:64, 0:1])
                nc.vector.tensor_scalar_add(
                    qT1[64:128, t0:t0 + TQ], ps[64:128, :], bqkv_sb[64:128, 0:1])
            elif m == 1:
                nc.vector.tensor_scalar_add(
                    kT_sb[:, t0:t0 + TQ], ps, bqkv_sb[:, 1:2])
            else:
                vt = vt_pool.tile([128, TQ], F32, name="vt", tag="vt")
                nc.vector.tensor_scalar_add(vt, ps, bqkv_sb[:, 2:3])
                for s in range(4):
                    n = t * 4 + s
                    tp = ps_sm.tile([128, 128], F32, name="tp", tag="sm")
                    # fp32 PE transpose (exact): [vdim, t]^T -> [t, vdim]
                    nc.tensor.transpose(tp, vt[:, s * 128:(s + 1) * 128], id_sb)
                    dst = v_all[:, n, :].rearrange(
                        "p (g e) -> p g e", g=2, e=65)[:, :, 0:64]
                    src = tp.rearrange("p (g e) -> p g e", g=2, e=64)
                    nc.vector.tensor_copy(dst, src)
            yield

    qkv_gens = [emit_qkv(t) for t in range(NI)]

    def drive(gen):
        if gen is not None:
            try:
                next(gen)
            except StopIteration:
                pass

    def finish(gen):
        if gen is not None:
            for _ in gen:
                pass

    finish(qkv_gens[0])  # i-tile 0 needed immediately

    # which QKV emission to interleave into each (half, h, i_t) block
    def gen_for(half, h, i_t):
        if half == 0 and h == 0 and i_t < 3:
            return qkv_gens[i_t + 1]
        if half == 0 and h == 1:
            return qkv_gens[4 + i_t]
        return None

    # --- attention: i_t-outer, J-pairs staged into one wide ACT ---
    pending_tail = [None]

    def flush_tail():
        if pending_tail[0] is not None:
            pending_tail[0]()
            pending_tail[0] = None

    yns = {}
    for half in range(2):
        for h in range(2):
            qT_h = qT0 if h == 0 else qT1
            for i_t in range(half * 4, half * 4 + 4):
                i0 = i_t * TQ
                nJ = 4 * i_t + 4
                gen = gen_for(half, h, i_t)
                o = ps_o.tile([128, TQ], F32, name="o_ps", tag="o")
                for Ja in range(0, nJ, 2):
                    drive(gen)
                    stage = ps_stage.tile([128, 1024], F32, name="st", tag="st")
                    metas, off = [], 0
                    for J in (Ja, Ja + 1):
                        r = max(0, J * TJ - i0)
                        w = TQ - r
                        nc.tensor.matmul(
                            stage[:, off:off + w],
                            lhsT=kT_sb[:, J * TJ:(J + 1) * TJ],
                            rhs=qT_h[:, i0 + r:i0 + TQ],
                            start=True, stop=True,
                        )
                        metas.append((J, r, off, w))
                        off += w
                    pt = pt_pool.tile([128, 1024], MM_DT, name="pt", tag="pt")
                    nc.scalar.activation(pt[:, 0:off], stage[:, 0:off], AF.Exp)
                    for (J, r, o_, w) in metas:
                        if J >= 4 * i_t:  # diag block: in-block triangle mask
                            nc.gpsimd.tensor_mul(
                                pt[:, o_:o_ + 128], pt[:, o_:o_ + 128], mask_sb)
                    for (J, r, o_, w) in metas:
                        v_lhs = (v_all[:, J, 0:65] if h == 0
                                 else v_all[:, J, 65:130])
                        nc.tensor.matmul(
                            o[0:65, r:TQ],
                            lhsT=v_lhs,
                            rhs=pt[:, o_:o_ + w],
                            start=(J == 0),
                            stop=(J == nJ - 1),
                        )
                    if Ja == 0:
                        flush_tail()
                finish(gen)

                def make_tail(h=h, i_t=i_t, o=o, i0=i0):
                    def tail():
                        # softmax division: broadcast denominator row via a
                        # one-hot matmul, reciprocal, multiply; evict O' from
                        # PSUM to SBUF first so the bank frees early.
                        o_sb = osb_pool.tile([128, TQ], MM_DT, name="o_sb",
                                             tag="osb")
                        nc.vector.tensor_copy(o_sb[0:65, :], o[0:65, :])
                        den = ps_sm.tile([128, TQ], F32, name="den", tag="sm")
                        nc.tensor.matmul(
                            den[0:64, :], lhsT=e64_sb[0:65, :],
                            rhs=o_sb[0:65, :], start=True, stop=True)
                        rc = rc_pool.tile([64, TQ], F32, name="rc", tag="rc")
                        nc.vector.reciprocal_approx_fast(out=rc, in_=den[0:64, :])
                        if h == 0:
                            yn = yn_pool.tile([128, TQ], MM_DT, name="yn",
                                              tag="yn")
                            yns[i_t] = yn
                        else:
                            yn = yns[i_t]
                        nc.vector.tensor_mul(
                            yn[h * 64:(h + 1) * 64, :], o_sb[0:64, :], rc)
                        if h == 1:
                            # output projection for this i_t (both heads)
                            for mc in range(4):
                                po = ps_sm.tile([128, TQ], F32, name="po",
                                                tag="sm")
                                nc.tensor.matmul(
                                    po, lhsT=wp_sb[:, mc * 128:(mc + 1) * 128],
                                    rhs=yn, start=True, stop=True)
                                ob = ob_pool.tile([128, TQ], BF16, name="ob",
                                                  tag="ob")
                                nc.vector.tensor_scalar_add(
                                    ob, po, bp_sb[:, mc:mc + 1])
                                nc.sync.dma_start(
                                    out=outT[mc * 128:(mc + 1) * 128,
                                             i0:i0 + TQ],
                                    in_=ob)
                    return tail

                pending_tail[0] = make_tail()
    flush_tail()


_CACHED_NC = None


def _build_program():
    global _CACHED_NC
    if _CACHED_NC is not None:
        return _CACHED_NC
    from contextlib import ExitStack
    nc = bacc.Bacc("TRN2", target_bir_lowering=False, debug=False,
                   num_devices=N_CORES)
    with tile.TileContext(nc) as tc:
        with ExitStack() as ctx:
            _emit(nc, tc, ctx)
    nc.compile()
    _CACHED_NC = nc
    return nc


def _host_inputs(x, w_attn, b_attn, w_proj, b_proj):
    """Build the 8 per-core input maps."""
    import ml_dtypes
    innp = ml_dtypes.bfloat16
    x = np.asarray(x, dtype=np.float32)
    w_attn = np.asarray(w_attn, dtype=np.float32)
    b_attn = np.asarray(b_attn, dtype=np.float32)
    w_proj = np.asarray(w_proj, dtype=np.float32)
    b_proj = np.asarray(b_proj, dtype=np.float32)

    scale = np.float32(1.0 / np.sqrt(D))
    mask = np.triu(np.ones((128, 128), dtype=np.float32))  # keep jj <= ii
    ident = np.eye(128, dtype=np.float32)
    e64 = np.zeros((128, 64), dtype=np.float32)
    e64[64, :] = 1.0

    xT_b = [np.ascontiguousarray(x[b].T).astype(innp) for b in range(B)]

    in_maps = []
    for core in range(N_CORES):
        b, hp = divmod(core, 4)
        r0 = 2 * hp * 64  # first row of this core's head-pair slice
        qr = w_attn[r0:r0 + 128] * scale
        kr = w_attn[C + r0:C + r0 + 128]
        vr = w_attn[2 * C + r0:2 * C + r0 + 128]
        wqkvT = np.ascontiguousarray(np.concatenate([qr, kr, vr], axis=0).T)
        bq = b_attn[r0:r0 + 128] * scale
        bk = b_attn[C + r0:C + r0 + 128]
        bv = b_attn[2 * C + r0:2 * C + r0 + 128]
        bqkv = np.ascontiguousarray(np.stack([bq, bk, bv], axis=1))
        wpT = np.ascontiguousarray(w_proj[:, r0:r0 + 128].T)
        if hp == 0:
            bpc = np.ascontiguousarray(b_proj.reshape(4, 128).T)
        else:
            bpc = np.zeros((128, 4), dtype=np.float32)
        in_maps.append({
            "xT": xT_b[b],
            "wqkvT": wqkvT.astype(innp),
            "bqkv": bqkv,
            "wpT": wpT.astype(np.float32),
            "bp": bpc,
            "mask01": mask,
            "ident": ident,
            "ones": np.ones((128, 1), dtype=np.float32),
            "e64": e64,
        })
    return in_maps


def _gather(results):
    out = np.empty((B, T, C), dtype=np.float32)
    for b in range(B):
        acc = results[b * 4]["outT"].astype(np.float32)
        for hp in range(1, 4):
            acc = acc + results[b * 4 + hp]["outT"].astype(np.float32)
        out[b] = acc.T
    return out


def kernel(x, w_attn, b_attn, w_proj, b_proj, _run_kwargs=None):
    nc = _build_program()
    in_maps = _host_inputs(x, w_attn, b_attn, w_proj, b_proj)
    kw = dict(_run_kwargs or {})
    res = bass_utils.run_bass_kernel_spmd(nc, in_maps,
                                          core_ids=list(range(N_CORES)), **kw)
    out = _gather(res.results)
    if _run_kwargs is not None:
        kernel.last_result = res
    return out
